# revision 7
# baseline (speedup 1.0000x reference)
"""Trainium2 Bass kernel for nn_CustomComputationLayer_87368224735398.

Computation per row (T=20 steps, input row = [c,b,i,rho]*20 ++ Irel*20):
  softmax over (c,b,i) -> c, b, sm2=1-c-b ; rho = sigmoid(rhoVal)
  wf_t = max(sm2 + 0.02 + rho*Irel, 1e-6)
  x_0 = 1 ; x_{t+1} = x_t * wf_t + 1
  total = sum_t[ (c_t x_t)^0.2 * pbard_t/(0.2*400) + (x_t(pd_t + b_t pd_t/(pd_t+1e-6)))^0.9 / 3600 ]
          + x_20^0.9 * pbard_20 / 3600

Sharding: pure data parallel over the batch dim across 8 cores.
Layout: rows-on-partitions, row-major free dim [p, (row, t)]; the T
recurrence runs as a single DVE tensor_tensor_scan along the free dim
with per-row boundary resets.

All clips except wf's lower bound are dead on randn inputs by wide
margins (verified numerically: c,b in [3e-4, 0.998], x <= 117,
pow args in [3.8e-4, 58]), so only max(wf, 1e-6) is emitted.
"""

import numpy as np

N_CORES = 8
BATCH = 1048576
D = 100
T = 20
P = 128

N_CORE = BATCH // N_CORES          # rows per core
ROWS_PER_PART = N_CORE // P        # 1024
NC_TILE = 64                       # rows per partition per tile
NTILES = ROWS_PER_PART // NC_TILE  # 16
W = NC_TILE * T                    # free width of per-(row,t) tiles

# ---- constants of the utility function (host-side) ----
_D_t = 0.995 ** np.arange(T + 1)
_pd_t = np.linspace(0.001, 0.01, T + 1)
_Pbard_t = 1.0 - np.cumsum(_pd_t)
PBARD = (_Pbard_t * _D_t).astype(np.float32)       # [T+1]
PD = (_pd_t * _D_t)[:-1].astype(np.float32)        # [T]
UTILITY_FACTOR = 1.0 / (T * T)
DELTA = 0.2
EPSILON = 0.9
K = 10.0
CB = (PBARD[:T] * (UTILITY_FACTOR / DELTA)).astype(np.float32)   # cons coeff per t
P1 = PD.astype(np.float32)
P2 = (PD / (PD + np.float32(1e-6))).astype(np.float32)
LEG_BIAS = float(np.log(UTILITY_FACTOR / (K * EPSILON)))
FIN_BIAS = float(np.log(PBARD[T] * UTILITY_FACTOR / (K * EPSILON)))
R_RATE = 0.02


def build_nc(n_core=N_CORE, nc_tile=NC_TILE):
    import concourse.bacc as bacc
    import concourse.tile as tile
    import concourse.mybir as mybir

    f32 = mybir.dt.float32
    AF = mybir.ActivationFunctionType
    OP = mybir.AluOpType

    rows_per_part = n_core // P
    ntiles = rows_per_part // nc_tile
    w = nc_tile * T

    nc = bacc.Bacc("TRN2", target_bir_lowering=False, debug=False)
    inp = nc.dram_tensor("inputs", [n_core, D], f32, kind="ExternalInput")
    cb_d = nc.dram_tensor("cbt", [P, w], f32, kind="ExternalInput")
    p1_d = nc.dram_tensor("p1t", [P, w], f32, kind="ExternalInput")
    p2_d = nc.dram_tensor("p2t", [P, w], f32, kind="ExternalInput")
    outp = nc.dram_tensor("out", [n_core, 1], f32, kind="ExternalOutput")

    in_r = inp.ap().rearrange("(p n) d -> p (n d)", p=P)    # [128, rows_per_part*100]
    out_r = outp.ap().rearrange("(p n) o -> p (n o)", p=P)  # [128, rows_per_part]

    with tile.TileContext(nc) as tc:
        with (
            tc.tile_pool(name="const", bufs=1) as constp,
            tc.tile_pool(name="io", bufs=2) as iop,
            tc.tile_pool(name="wk", bufs=1) as wk,
            tc.tile_pool(name="sm", bufs=2) as smp,
        ):
            cbt = constp.tile([P, w], f32, tag="cbt")
            nc.sync.dma_start(cbt[:], cb_d.ap())
            p1t = constp.tile([P, w], f32, tag="p1t")
            nc.sync.dma_start(p1t[:], p1_d.ap())
            p2t = constp.tile([P, w], f32, tag="p2t")
            nc.sync.dma_start(p2t[:], p2_d.ap())
            ones = constp.tile([P, w], f32, tag="ones")
            nc.vector.memset(ones[:], 1.0)
            bleg = constp.tile([P, 1], f32, tag="bleg")
            nc.vector.memset(bleg[:], LEG_BIAS)
            bfin = constp.tile([P, 1], f32, tag="bfin")
            nc.vector.memset(bfin[:], FIN_BIAS)

            for i in range(ntiles):
                raw = iop.tile([P, nc_tile * D], f32, tag="raw")
                nc.sync.dma_start(
                    raw[:], in_r[:, i * nc_tile * D : (i + 1) * nc_tile * D]
                )
                raw3 = raw[:].rearrange("p (n d) -> p n d", d=D)
                cv = raw3[:, :, 0 : 4 * T : 4]
                bv = raw3[:, :, 1 : 4 * T : 4]
                iv = raw3[:, :, 2 : 4 * T : 4]
                rv = raw3[:, :, 3 : 4 * T : 4]
                irel = raw3[:, :, 4 * T :]

                # exps (all on one ACT table set)
                e1 = wk.tile([P, w], f32, tag="e1")   # e^cv -> c
                e2 = wk.tile([P, w], f32, tag="e2")   # e^bv -> b
                e3 = wk.tile([P, w], f32, tag="e3")   # e^iv -> sm2
                ed = wk.tile([P, w], f32, tag="ed")   # e^-rv -> rho -> q -> wA
                nc.scalar.activation(e1[:], cv, AF.Exp)
                nc.scalar.activation(e2[:], bv, AF.Exp)
                nc.scalar.activation(e3[:], iv, AF.Exp)
                nc.scalar.activation(ed[:], rv, AF.Exp, scale=-1.0)

                s = wk.tile([P, w], f32, tag="s")
                nc.vector.tensor_add(s[:], e1[:], e2[:])
                nc.vector.tensor_add(s[:], s[:], e3[:])
                r = wk.tile([P, w], f32, tag="r")
                nc.vector.reciprocal_approx_fast(out=r[:], in_=s[:])

                # rho = 1/(1 + e^-rv), in place over ed
                nc.vector.tensor_scalar_add(ed[:], ed[:], 1.0)
                nc.vector.reciprocal_approx_fast(out=ed[:], in_=ed[:])

                nc.vector.tensor_mul(e1[:], e1[:], r[:])   # c
                nc.vector.tensor_mul(e2[:], e2[:], r[:])   # b
                nc.vector.tensor_mul(e3[:], e3[:], r[:])   # sm2
                nc.vector.tensor_mul(ed[:], ed[:], irel)   # q = rho*Irel
                nc.vector.tensor_add(ed[:], ed[:], e3[:])  # wA = q + sm2

                # wf = max(wA + 0.02, 1e-6), written shifted into the scan
                # multiplier A at [1 : w+1]; per-row boundary A[r*20]=0 gives
                # out[r*20] = 0*state + 1 = x_0.
                A = wk.tile([P, w + 1], f32, tag="A")
                nc.vector.tensor_scalar(
                    A[:, 1 : w + 1], ed[:], R_RATE, 1e-6, OP.add, OP.max
                )
                # save wf_19 per row before zeroing boundaries
                w19 = smp.tile([P, nc_tile], f32, tag="w19")
                nc.gpsimd.tensor_copy(w19[:], A[:, 20 : w + 1 : 20])
                nc.gpsimd.memset(A[:, 0:w:20], 0.0)

                xprev = wk.tile([P, w], f32, tag="xprev")
                nc.vector.tensor_tensor_scan(
                    xprev[:], A[:, 0:w], ones[:], 0.0, OP.mult, OP.add
                )
                x3 = xprev[:].rearrange("p (n t) -> p n t", t=20)

                # x20 = x19*wf19 + 1
                x20 = smp.tile([P, nc_tile], f32, tag="x20")
                nc.vector.tensor_mul(x20[:], x3[:, :, 19], w19[:])
                nc.vector.tensor_scalar_add(x20[:], x20[:], 1.0)
                # final term: x20^0.9 * pbard_T / 3600
                fterm = smp.tile([P, nc_tile], f32, tag="fterm")
                nc.scalar.activation(fterm[:], x20[:], AF.Ln)
                nc.scalar.activation(fterm[:], fterm[:], AF.Exp, bias=bfin[:], scale=EPSILON)

                # consumption: (c*x)^0.2 * cbt
                m = wk.tile([P, w], f32, tag="m")
                nc.vector.tensor_mul(m[:], e1[:], xprev[:])
                nc.scalar.activation(m[:], m[:], AF.Ln)
                nc.scalar.activation(m[:], m[:], AF.Exp, scale=DELTA)
                nc.vector.tensor_mul(m[:], m[:], cbt[:])

                # legacy: (x*(p1 + b*p2))^0.9 / 3600
                lb = wk.tile([P, w], f32, tag="lb")
                nc.vector.tensor_mul(lb[:], e2[:], p2t[:])
                nc.vector.tensor_add(lb[:], lb[:], p1t[:])
                nc.vector.tensor_mul(lb[:], lb[:], xprev[:])
                nc.scalar.activation(lb[:], lb[:], AF.Ln)
                nc.scalar.activation(lb[:], lb[:], AF.Exp, bias=bleg[:], scale=EPSILON)

                nc.vector.tensor_add(m[:], m[:], lb[:])
                tot3 = m[:].rearrange("p (n t) -> p n t", t=20)
                rowsum = smp.tile([P, nc_tile], f32, tag="rowsum")
                nc.vector.reduce_sum(rowsum[:], tot3, axis=mybir.AxisListType.X)
                nc.vector.tensor_add(rowsum[:], rowsum[:], fterm[:])

                nc.sync.dma_start(
                    out_r[:, i * nc_tile : (i + 1) * nc_tile], rowsum[:]
                )

    nc.compile()
    return nc


def _const_inputs(nc_tile=NC_TILE):
    w = nc_tile * T
    cbt = np.ascontiguousarray(np.tile(CB, (P, nc_tile)))
    p1t = np.ascontiguousarray(np.tile(P1, (P, nc_tile)))
    p2t = np.ascontiguousarray(np.tile(P2, (P, nc_tile)))
    assert cbt.shape == (P, w)
    return cbt.astype(np.float32), p1t.astype(np.float32), p2t.astype(np.float32)


def run(inputs: np.ndarray, trace: bool = False, tmpdir=None):
    from concourse.bass_utils import run_bass_kernel_spmd

    x = np.ascontiguousarray(np.asarray(inputs, dtype=np.float32))
    assert x.shape == (BATCH, D)
    nc = build_nc()
    cbt, p1t, p2t = _const_inputs()
    in_maps = []
    for k in range(N_CORES):
        in_maps.append(
            {
                "inputs": np.ascontiguousarray(x[k * N_CORE : (k + 1) * N_CORE]),
                "cbt": cbt,
                "p1t": p1t,
                "p2t": p2t,
            }
        )
    res = run_bass_kernel_spmd(
        nc, in_maps, list(range(N_CORES)), trace=trace, tmpdir=tmpdir
    )
    out = np.concatenate([res.results[k]["out"] for k in range(N_CORES)], axis=0)
    return out.astype(np.float32), res


def kernel(**inputs) -> np.ndarray:
    out, _ = run(inputs["inputs"], trace=False)
    return out


def run_timed(inputs: np.ndarray, n_iters: int = 10):
    """Execute via PJRT with device-resident inputs and wall-clock the
    jitted executable (the axon env has no NTFF profiling hook)."""
    import jax
    import jax.numpy as jnp
    from jax.sharding import Mesh, PartitionSpec, NamedSharding
    from jax.experimental.shard_map import shard_map
    import concourse.mybir as mybir
    from concourse import bass2jax
    from concourse.bass2jax import _bass_exec_p, install_neuronx_cc_hook
    import time

    install_neuronx_cc_hook()
    x = np.ascontiguousarray(np.asarray(inputs, dtype=np.float32))
    nc = build_nc()
    cbt, p1t, p2t = _const_inputs()
    full_ins = {
        "inputs": x,
        "cbt": np.concatenate([cbt] * N_CORES, 0),
        "p1t": np.concatenate([p1t] * N_CORES, 0),
        "p2t": np.concatenate([p2t] * N_CORES, 0),
    }

    partition_name = nc.partition_id_tensor.name if nc.partition_id_tensor else None
    in_names, out_names, out_avals, zero_outs = [], [], [], []
    for alloc in nc.m.functions[0].allocations:
        if not isinstance(alloc, mybir.MemoryLocationSet):
            continue
        name = alloc.memorylocations[0].name
        if alloc.kind == "ExternalInput":
            if name != partition_name:
                in_names.append(name)
        elif alloc.kind == "ExternalOutput":
            out_names.append(name)
            shape = tuple(alloc.tensor_shape)
            dtype = mybir.dt.np(alloc.dtype)
            out_avals.append(jax.core.ShapedArray(shape, dtype))
            zero_outs.append(np.zeros((N_CORES * shape[0], *shape[1:]), dtype))
    n_params = len(in_names)
    all_in_names = list(in_names) + list(out_names)
    if partition_name is not None:
        all_in_names.append(partition_name)

    def _body(*args):
        operands = list(args)
        if partition_name is not None:
            operands.append(bass2jax.partition_id_tensor())
        outs = _bass_exec_p.bind(
            *operands,
            out_avals=tuple(out_avals),
            in_names=tuple(all_in_names),
            out_names=tuple(out_names),
            lowering_input_output_aliases=(),
            sim_require_finite=True,
            sim_require_nnan=True,
            nc=nc,
        )
        return tuple(outs)

    devices = jax.devices()[:N_CORES]
    mesh = Mesh(np.asarray(devices), ("core",))
    spec = PartitionSpec("core")
    in_specs = (spec,) * (n_params + len(out_names))
    out_specs = (spec,) * len(out_names)
    fn = jax.jit(
        shard_map(_body, mesh=mesh, in_specs=in_specs, out_specs=out_specs,
                  check_rep=False),
        keep_unused=True,
    )
    sh = NamedSharding(mesh, spec)
    dev_args = [jax.device_put(full_ins[n], sh) for n in in_names]
    dev_zeros = [jax.device_put(z, sh) for z in zero_outs]

    out = fn(*dev_args, *dev_zeros)
    jax.block_until_ready(out)
    times = []
    for _ in range(n_iters):
        t0 = time.perf_counter()
        out = fn(*dev_args, *dev_zeros)
        jax.block_until_ready(out)
        times.append(time.perf_counter() - t0)
    out_np = np.asarray(out[0]).reshape(BATCH, 1).astype(np.float32)
    return out_np, times


# revision 31
# speedup vs baseline: 233.0872x; 233.0872x over previous
"""Trainium2 Bass kernel for nn_CustomComputationLayer_87368224735398.

Computation per row (T=20 steps, input row = [c,b,i,rho]*20 ++ Irel*20):
  softmax over (c,b,i) -> c, b, sm2=1-c-b ; rho = sigmoid(rhoVal)
  wf_t = max(sm2 + 0.02 + rho*Irel, 1e-6)
  x_0 = 1 ; x_{t+1} = x_t * wf_t + 1
  total = sum_t[ (c_t x_t)^0.2 * pbard_t/(0.2*400) + (x_t(pd_t + b_t pd_t/(pd_t+1e-6)))^0.9 / 3600 ]
          + x_20^0.9 * pbard_20 / 3600

Sharding: pure data parallel over the batch dim across 8 cores.
Layout: rows-on-partitions, row-major free dim [p, (row, t)]; the T
recurrence runs as a single DVE tensor_tensor_scan along the free dim
with per-row boundary resets.

All clips except wf's lower bound are dead on randn inputs by wide
margins (verified numerically: c,b in [3e-4, 0.998], x <= 117,
pow args in [3.8e-4, 58]), so only max(wf, 1e-6) is emitted.
"""

import numpy as np

N_CORES = 8
BATCH = 1048576
D = 100
T = 20
P = 128

N_CORE = BATCH // N_CORES          # rows per core
ROWS_PER_PART = N_CORE // P        # 1024
NC_TILE = 64                       # rows per partition per tile
NTILES = ROWS_PER_PART // NC_TILE  # 16
W = NC_TILE * T                    # free width of per-(row,t) tiles

# ---- constants of the utility function (host-side) ----
_D_t = 0.995 ** np.arange(T + 1)
_pd_t = np.linspace(0.001, 0.01, T + 1)
_Pbard_t = 1.0 - np.cumsum(_pd_t)
PBARD = (_Pbard_t * _D_t).astype(np.float32)       # [T+1]
PD = (_pd_t * _D_t)[:-1].astype(np.float32)        # [T]
UTILITY_FACTOR = 1.0 / (T * T)
DELTA = 0.2
EPSILON = 0.9
K = 10.0
CB = (PBARD[:T] * (UTILITY_FACTOR / DELTA)).astype(np.float32)   # cons coeff per t
P1 = PD.astype(np.float32)
P2 = (PD / (PD + np.float32(1e-6))).astype(np.float32)
LEG_BIAS = float(np.log(UTILITY_FACTOR / (K * EPSILON)))
FIN_BIAS = float(np.log(PBARD[T] * UTILITY_FACTOR / (K * EPSILON)))
R_RATE = 0.02


def _make_bacc_class():
    """Bacc subclass that pins Exp+Ln to the one table set containing both
    (natural_log_exp_and_others), avoiding a ~2.7us ACT table reload between
    every Ln<->Exp transition (2 per tile ~= 85us/core otherwise)."""
    import bass_rust as _bass_rust
    import concourse.bacc as bacc
    import concourse.mybir as mybir
    from concourse.hw_specs import get_activation_tables

    AF = mybir.ActivationFunctionType

    class EconBacc(bacc.Bacc):
        def insert_act_table_loads(self):
            has_activation = any(
                isinstance(i, mybir.InstActivation)
                for b in self.main_func.blocks
                for i in b.instructions
            )
            if not has_activation:
                return
            tables = []
            for name, funcs in get_activation_tables(self.m.arch).items():
                if name != "natural_log_exp_and_others":
                    funcs = funcs - {AF.Exp, AF.Ln}
                tables.append((name, funcs))
            _bass_rust.insert_act_table_loads(self, tables)

    return EconBacc


def build_nc(n_core=N_CORE, nc_tile=NC_TILE, reps=1, wk_bufs=3, io_bufs=2,
             use_sigmoid=True, psum_xprev=True):
    import concourse.bacc as bacc
    import concourse.bass as bass
    import concourse.tile as tile
    import concourse.mybir as mybir

    f32 = mybir.dt.float32
    AF = mybir.ActivationFunctionType
    OP = mybir.AluOpType

    rows_per_part = n_core // P
    ntiles = rows_per_part // nc_tile
    w = nc_tile * T

    nc = _make_bacc_class()("TRN2", target_bir_lowering=False, debug=False)
    inp = nc.dram_tensor("inputs", [n_core, D], f32, kind="ExternalInput")
    cb_d = nc.dram_tensor("cbt", [P, T], f32, kind="ExternalInput")
    p1_d = nc.dram_tensor("p1t", [P, T], f32, kind="ExternalInput")
    p2_d = nc.dram_tensor("p2t", [P, T], f32, kind="ExternalInput")
    outp = nc.dram_tensor("out", [n_core, 1], f32, kind="ExternalOutput")

    in_r = inp.ap().rearrange("(p n) d -> p (n d)", p=P)    # [128, rows_per_part*100]
    out_r = outp.ap().rearrange("(p n) o -> p (n o)", p=P)  # [128, rows_per_part]

    with tile.TileContext(nc) as tc:
        with (
            tc.tile_pool(name="const", bufs=1) as constp,
            tc.tile_pool(name="io", bufs=io_bufs) as iop,
            tc.tile_pool(name="wk", bufs=wk_bufs) as wk,
            tc.tile_pool(name="wk2", bufs=max(2, wk_bufs)) as wk2,
            tc.tile_pool(name="sm", bufs=2) as smp,
            tc.tile_pool(name="ps", bufs=1, space="PSUM") as psp,
        ):
            def bcast(ct, n):
                # [P, T] tile viewed as [P, n, T] with a 0-stride row dim
                a = ct[:]
                return bass.AP(a.tensor, a.offset, [a.ap[0], [0, n], [1, T]])

            cbt = constp.tile([P, T], f32, tag="cbt")
            nc.sync.dma_start(cbt[:], cb_d.ap())
            p1t = constp.tile([P, T], f32, tag="p1t")
            nc.sync.dma_start(p1t[:], p1_d.ap())
            p2t = constp.tile([P, T], f32, tag="p2t")
            nc.sync.dma_start(p2t[:], p2_d.ap())
            onep = constp.tile([P, 1], f32, tag="onep")
            nc.vector.memset(onep[:], 1.0)
            _oa = onep[:]
            ones_b = bass.AP(_oa.tensor, _oa.offset, [_oa.ap[0], [0, w]])
            cb_b = bcast(cbt, nc_tile)
            p1_b = bcast(p1t, nc_tile)
            p2_b = bcast(p2t, nc_tile)
            bleg = constp.tile([P, 1], f32, tag="bleg")
            nc.vector.memset(bleg[:], LEG_BIAS)
            bfin = constp.tile([P, 1], f32, tag="bfin")
            nc.vector.memset(bfin[:], FIN_BIAS)

            def _tile_body():
             for i in range(ntiles):
                raw = iop.tile([P, nc_tile * D], f32, tag="raw")
                nc.sync.dma_start(
                    raw[:], in_r[:, i * nc_tile * D : (i + 1) * nc_tile * D]
                )
                raw3 = raw[:].rearrange("p (n d) -> p n d", d=D)
                cv = raw3[:, :, 0 : 4 * T : 4]
                bv = raw3[:, :, 1 : 4 * T : 4]
                iv = raw3[:, :, 2 : 4 * T : 4]
                rv = raw3[:, :, 3 : 4 * T : 4]
                irel = raw3[:, :, 4 * T :]

                # exps (all on one ACT table set)
                e1 = wk2.tile([P, w], f32, tag="e1")   # e^cv -> c
                e2 = wk2.tile([P, w], f32, tag="e2")   # e^bv -> b
                e3 = wk.tile([P, w], f32, tag="e3")   # e^iv -> c+b scratch
                ed = wk.tile([P, w], f32, tag="ed")   # rho -> q -> wA
                if use_sigmoid:
                    # own table set; placed first so each tile pays exactly
                    # two set switches (sig -> ln/exp)
                    nc.scalar.activation(ed[:], rv, AF.Sigmoid)
                nc.scalar.activation(e1[:], cv, AF.Exp)
                nc.scalar.activation(e2[:], bv, AF.Exp)
                nc.scalar.activation(e3[:], iv, AF.Exp)
                if not use_sigmoid:
                    nc.scalar.activation(ed[:], rv, AF.Exp, scale=-1.0)

                s = wk.tile([P, w], f32, tag="s")
                nc.vector.tensor_add(s[:], e1[:], e2[:])
                nc.vector.tensor_add(s[:], s[:], e3[:])
                r = wk.tile([P, w], f32, tag="r")
                nc.vector.reciprocal_approx_fast(out=r[:], in_=s[:])

                if not use_sigmoid:
                    # rho = 1/(1 + e^-rv), in place over ed
                    nc.vector.tensor_scalar_add(ed[:], ed[:], 1.0)
                    nc.vector.reciprocal_approx_fast(out=ed[:], in_=ed[:])

                nc.vector.tensor_mul(e1[:], e1[:], r[:])   # c
                nc.vector.tensor_mul(e2[:], e2[:], r[:])   # b
                nc.vector.tensor_mul(ed[:], ed[:], irel)   # q = rho*Irel
                # sm2 = 1-c-b, so wf-pre = sm2 + 0.02 + q = (q - (c+b)) + 1.02
                nc.vector.tensor_add(e3[:], e1[:], e2[:])  # c + b (reuses ei buf)
                nc.vector.scalar_tensor_tensor(
                    ed[:], e3[:], -1.0, ed[:], OP.mult, OP.add
                )  # q - (c+b)

                # wf = max(wA + 1.02, 1e-6), written shifted into the scan
                # multiplier A at [1 : w+1]; per-row boundary A[r*20]=0 gives
                # out[r*20] = 0*state + 1 = x_0.
                A = wk.tile([P, w + 1], f32, tag="r")
                nc.vector.tensor_scalar(
                    A[:, 1 : w + 1], ed[:], 1.0 + R_RATE, 1e-6, OP.add, OP.max
                )
                # save wf_19 per row before zeroing boundaries
                w19 = smp.tile([P, nc_tile], f32, tag="w19")
                nc.gpsimd.tensor_copy(w19[:], A[:, 20 : w + 1 : 20])
                nc.gpsimd.memset(A[:, 0:w:20], 0.0)

                if psum_xprev:
                    xprev = psp.tile([P, w], f32, tag="xprev")
                else:
                    xprev = wk.tile([P, w], f32, tag="xprev")
                nc.vector.tensor_tensor_scan(
                    xprev[:], A[:, 0:w], ones_b, 0.0, OP.mult, OP.add
                )
                x3 = xprev[:].rearrange("p (n t) -> p n t", t=20)

                # x20 = x19*wf19 + 1
                x20 = smp.tile([P, nc_tile], f32, tag="x20")
                nc.vector.tensor_mul(x20[:], x3[:, :, 19], w19[:])
                nc.vector.tensor_scalar_add(x20[:], x20[:], 1.0)
                # final term: x20^0.9 * pbard_T / 3600
                fterm = smp.tile([P, nc_tile], f32, tag="fterm")
                nc.scalar.activation(fterm[:], x20[:], AF.Ln)
                nc.scalar.activation(fterm[:], fterm[:], AF.Exp, bias=bfin[:], scale=EPSILON)

                # consumption: (c*x)^0.2 * cbt   (reuses sm2's buffer)
                m = wk.tile([P, w], f32, tag="e3")
                nc.vector.tensor_mul(m[:], e1[:], xprev[:])
                nc.scalar.activation(m[:], m[:], AF.Ln)
                nc.scalar.activation(m[:], m[:], AF.Exp, scale=DELTA)
                m3 = m[:].rearrange("p (n t) -> p n t", t=T)
                nc.vector.tensor_tensor(m3, m3, cb_b, OP.mult)

                # legacy: (x*(p1 + b*p2))^0.9 / 3600   (reuses s's buffer)
                lb = wk.tile([P, w], f32, tag="s")
                lb3 = lb[:].rearrange("p (n t) -> p n t", t=T)
                e23 = e2[:].rearrange("p (n t) -> p n t", t=T)
                nc.vector.tensor_tensor(lb3, e23, p2_b, OP.mult)
                nc.vector.tensor_tensor(lb3, lb3, p1_b, OP.add)
                nc.vector.tensor_mul(lb[:], lb[:], xprev[:])
                nc.scalar.activation(lb[:], lb[:], AF.Ln)
                nc.scalar.activation(lb[:], lb[:], AF.Exp, bias=bleg[:], scale=EPSILON)

                nc.vector.tensor_add(m[:], m[:], lb[:])
                tot3 = m[:].rearrange("p (n t) -> p n t", t=20)
                rowsum = smp.tile([P, nc_tile], f32, tag="rowsum")
                nc.vector.reduce_sum(rowsum[:], tot3, axis=mybir.AxisListType.X)
                nc.vector.tensor_add(rowsum[:], rowsum[:], fterm[:])

                nc.sync.dma_start(
                    out_r[:, i * nc_tile : (i + 1) * nc_tile], rowsum[:]
                )

            if reps > 1:
                with tc.For_i(0, reps, 1):
                    _tile_body()
            else:
                _tile_body()

    nc.compile()
    return nc


def _const_inputs(nc_tile=NC_TILE):
    cbt = np.ascontiguousarray(np.tile(CB, (P, 1)))
    p1t = np.ascontiguousarray(np.tile(P1, (P, 1)))
    p2t = np.ascontiguousarray(np.tile(P2, (P, 1)))
    assert cbt.shape == (P, T)
    return cbt.astype(np.float32), p1t.astype(np.float32), p2t.astype(np.float32)


def run(inputs: np.ndarray, trace: bool = False, tmpdir=None):
    from concourse.bass_utils import run_bass_kernel_spmd

    x = np.ascontiguousarray(np.asarray(inputs, dtype=np.float32))
    assert x.shape == (BATCH, D)
    nc = build_nc()
    cbt, p1t, p2t = _const_inputs()
    in_maps = []
    for k in range(N_CORES):
        in_maps.append(
            {
                "inputs": np.ascontiguousarray(x[k * N_CORE : (k + 1) * N_CORE]),
                "cbt": cbt,
                "p1t": p1t,
                "p2t": p2t,
            }
        )
    res = run_bass_kernel_spmd(
        nc, in_maps, list(range(N_CORES)), trace=trace, tmpdir=tmpdir
    )
    out = np.concatenate([res.results[k]["out"] for k in range(N_CORES)], axis=0)
    return out.astype(np.float32), res


def kernel(**inputs) -> np.ndarray:
    out, _ = run(inputs["inputs"], trace=False)
    return out


def run_timed(inputs: np.ndarray, n_iters: int = 10):
    """Execute via PJRT with device-resident inputs and wall-clock the
    jitted executable (the axon env has no NTFF profiling hook)."""
    import jax
    import jax.numpy as jnp
    from jax.sharding import Mesh, PartitionSpec, NamedSharding
    from jax.experimental.shard_map import shard_map
    import concourse.mybir as mybir
    from concourse import bass2jax
    from concourse.bass2jax import _bass_exec_p, install_neuronx_cc_hook
    import time

    install_neuronx_cc_hook()
    x = np.ascontiguousarray(np.asarray(inputs, dtype=np.float32))
    nc = build_nc()
    cbt, p1t, p2t = _const_inputs()
    full_ins = {
        "inputs": x,
        "cbt": np.concatenate([cbt] * N_CORES, 0),
        "p1t": np.concatenate([p1t] * N_CORES, 0),
        "p2t": np.concatenate([p2t] * N_CORES, 0),
    }

    partition_name = nc.partition_id_tensor.name if nc.partition_id_tensor else None
    in_names, out_names, out_avals, zero_outs = [], [], [], []
    for alloc in nc.m.functions[0].allocations:
        if not isinstance(alloc, mybir.MemoryLocationSet):
            continue
        name = alloc.memorylocations[0].name
        if alloc.kind == "ExternalInput":
            if name != partition_name:
                in_names.append(name)
        elif alloc.kind == "ExternalOutput":
            out_names.append(name)
            shape = tuple(alloc.tensor_shape)
            dtype = mybir.dt.np(alloc.dtype)
            out_avals.append(jax.core.ShapedArray(shape, dtype))
            zero_outs.append(np.zeros((N_CORES * shape[0], *shape[1:]), dtype))
    n_params = len(in_names)
    all_in_names = list(in_names) + list(out_names)
    if partition_name is not None:
        all_in_names.append(partition_name)

    def _body(*args):
        operands = list(args)
        if partition_name is not None:
            operands.append(bass2jax.partition_id_tensor())
        outs = _bass_exec_p.bind(
            *operands,
            out_avals=tuple(out_avals),
            in_names=tuple(all_in_names),
            out_names=tuple(out_names),
            lowering_input_output_aliases=(),
            sim_require_finite=True,
            sim_require_nnan=True,
            nc=nc,
        )
        return tuple(outs)

    devices = jax.devices()[:N_CORES]
    mesh = Mesh(np.asarray(devices), ("core",))
    spec = PartitionSpec("core")
    in_specs = (spec,) * (n_params + len(out_names))
    out_specs = (spec,) * len(out_names)
    fn = jax.jit(
        shard_map(_body, mesh=mesh, in_specs=in_specs, out_specs=out_specs,
                  check_rep=False),
        keep_unused=True,
    )
    sh = NamedSharding(mesh, spec)
    dev_args = [jax.device_put(full_ins[n], sh) for n in in_names]
    dev_zeros = [jax.device_put(z, sh) for z in zero_outs]

    out = fn(*dev_args, *dev_zeros)
    jax.block_until_ready(out)
    times = []
    for _ in range(n_iters):
        t0 = time.perf_counter()
        out = fn(*dev_args, *dev_zeros)
        jax.block_until_ready(out)
        times.append(time.perf_counter() - t0)
    out_np = np.asarray(out[0]).reshape(BATCH, 1).astype(np.float32)
    return out_np, times
